# revision 75
# baseline (speedup 1.0000x reference)
"""Trainium2 Bass kernel for the sparse-attention nn.Module.

Math: with use_advanced=1, dots = cos_w*cosine + const(h,b) where const is a
per-(head,batch) scalar from chunk statistics, so attention factorizes into
linear attention:
  f_x = LN(x) @ (diag(ln_gamma) W_in)          x in {q,k,v}
  KV  = (f_k/|f_k|)^T f_v                      [d,d] per (h,b)
  out = (cos_w/|f_q|) f_q @ KV + const(h,b) * colsum(f_v)
  y   = out @ W_out + b_out
Chunk stats via per-sample Gram matrices G_c = f^T f - musum musum^T / N.
One chunk == one (head, core) pair when B=64 is sharded 8-way over cores.

Schedule (v3): LN's elementwise apply is folded into the projection —
only (x - mu) is materialized (cheap 4x-mode tensor_scalar), transposed,
and the per-row rstd rides in as a per-partition scale during PSUM
evacuation.  Gram/KV reduction work is software-pipelined one b behind
the matmul windows so the PE never head-of-line blocks on Act/DVE.  The
per-(h,b) additive constant is applied as a rank-1 matmul (ones x yrow)
inside the output-projection accumulation, taking the stats chain off
the S-evacuation path.  b_out is folded into the same rank-1 row.
"""
import math
import numpy as np
import ml_dtypes

import concourse.bacc as bacc
import concourse.bass as bass
import concourse.bass_isa as bass_isa
import concourse.mybir as mybir
import concourse.tile as tile
from concourse.tile_rust import add_dep_helper
from concourse.bass_utils import run_bass_kernel_spmd

F32, BF16 = mybir.dt.float32, mybir.dt.bfloat16
AF = mybir.ActivationFunctionType
ALU = mybir.AluOpType
AX = mybir.AxisListType

B, N, D = 64, 256, 512
H, d = 8, 64
NCORES = 8
Bs = B // NCORES            # 8 batches per core
R = Bs * N                  # 2048 rows per core
NT = R // 128               # 16 row tiles
LN_EPS = 1e-5
EPS = 1e-8

_CACHE = {}


def _build(cos_w, cov_w, var_w):
    sgn = 1.0 if cos_w >= 0 else -1.0
    q_rs = 1.0 / (cos_w * cos_w)    # Rsqrt input scale folding |cos_w|
    nc = bacc.Bacc("TRN2", target_bir_lowering=False, debug=False,
                   num_devices=NCORES)
    xd = {t: nc.dram_tensor(f"x{t}", [R, D], BF16, kind="ExternalInput")
          for t in "qkv"}
    wp_d = nc.dram_tensor("wp", [D, D], BF16, kind="ExternalInput")
    wo_d = nc.dram_tensor("wo", [D, D], BF16, kind="ExternalInput")
    bob_d = nc.dram_tensor("bob", [1, D], BF16, kind="ExternalInput")
    e8b_d = nc.dram_tensor("e8b", [128, 64], BF16, kind="ExternalInput")
    y_d = nc.dram_tensor("y", [R, D], BF16, kind="ExternalOutput")
    # DRAM scratch for layout bounces
    stb_d = nc.dram_tensor("stb_scr", [4, Bs, D], BF16)   # muq muk negq negk
    wb_d = nc.dram_tensor("wb_scr", [Bs, D], BF16)        # wbs rows
    ya_d = nc.dram_tensor("ya_scr", [Bs, D], BF16)        # yadd rows

    with tile.TileContext(nc) as tc:
        with (
            tc.tile_pool(name="cst", bufs=1) as cst,
            tc.tile_pool(name="big", bufs=1) as big,
            tc.tile_pool(name="zt", bufs=2) as ztp,
            tc.tile_pool(name="ln", bufs=3) as lnp,
            tc.tile_pool(name="st3", bufs=2) as st3,
            tc.tile_pool(name="st2", bufs=2) as st2,
            tc.tile_pool(name="sml", bufs=3) as sml,
            tc.tile_pool(name="ps", bufs=2, space=bass.MemorySpace.PSUM) as ps,
            tc.tile_pool(name="pss", bufs=2, space=bass.MemorySpace.PSUM) as pss,
            tc.tile_pool(name="ps3", bufs=3, space=bass.MemorySpace.PSUM) as ps3,
            tc.tile_pool(name="ps1", bufs=1, space=bass.MemorySpace.PSUM) as ps1,
        ):
            # ---- constants
            wp = cst.tile([128, 4, D], BF16, tag="wp")
            wo = cst.tile([128, 4, D], BF16, tag="wo")
            bob = cst.tile([1, D], BF16, tag="bob")
            e8b = cst.tile([128, 64], BF16, tag="e8b")
            epsb = cst.tile([128, 1], F32, tag="epsb")
            nc.gpsimd.memset(epsb[:], LN_EPS)
            epsc = cst.tile([8, 1], F32, tag="epsc")
            nc.gpsimd.memset(epsc[:], EPS)
            one8 = cst.tile([8, 1], F32, tag="one8")
            nc.gpsimd.memset(one8[:], 1.0)
            ones1 = cst.tile([1, 128], BF16, tag="ones1")
            nc.gpsimd.memset(ones1[:], 1.0)

            def emit_consts():
                nc.scalar.dma_start(
                    wp[:], wp_d[:].rearrange("(c p) o -> p c o", p=128))
                nc.scalar.dma_start(
                    wo[:], wo_d[:].rearrange("(c p) o -> p c o", p=128))
                nc.scalar.dma_start(bob[:], bob_d[:])
                nc.scalar.dma_start(e8b[:], e8b_d[:])

            # ---- resident activations
            fqT = big.tile([128, 16, R // 4], BF16, tag="fqT")
            fkn = big.tile([128, NT, D], BF16, tag="fkn")
            fv = big.tile([128, NT, D], BF16, tag="fv")
            fres = {"q": big.tile([128, NT, D], BF16, tag="fresq", name="fresq"),
                    "k": big.tile([128, NT, D], BF16, tag="fresk", name="fresk")}
            stat48 = cst.tile([8, 48], F32, tag="stat48")
            kvblk = big.tile([128, Bs, 4, 128], BF16, tag="kvblk")
            # per-(tensor,tile) rstd scales and per-row means
            rstd_all = cst.tile([128, 3, NT], F32, tag="rstd")
            TIX = {"q": 0, "k": 1, "v": 2}

            # PSUM stat bank (matmul outs must start at partition 0/32/64/96);
            # slices time-share the bank.
            statQK = ps1.tile([128, D], F32, tag="statQK")
            mu_ps = {"q": statQK[0:8, :], "k": statQK[64:72, :]}
            cs_ps = {"q": statQK[32:40, :], "k": statQK[0:8, :]}
            vsum_ps = statQK[64:72, :]
            gsq_ps = {"q": statQK[0:8, :], "k": statQK[32:40, :]}
            yadd_ps = statQK[64:72, :]

            # ---- PE warm-up: dummy matmuls absorb the cold p-state window
            # while the first DMA loads + LN chain run, so real projection
            # matmuls issue against a warmed clock with no leading idle gap.
            warm_w = cst.tile([128, 128], BF16, tag="warmw")
            nc.gpsimd.memset(warm_w[:], 0.0)
            warm_r = cst.tile([128, D], BF16, tag="warmr")
            nc.gpsimd.memset(warm_r[:], 0.0)
            for wi in range(9):
                wps = ps3.tile([128, D], F32, tag="pp", name=f"warm{wi}")
                for fc in range(3):
                    nc.tensor.matmul(wps[:], warm_w[:], warm_r[:],
                                     start=(fc == 0), stop=(fc == 2))

            # ====== Pipelined schedule ======================================
            xcTs, xq4s, strow, mub_keep = {}, {}, {}, {}

            def emit_xload(tname, tg):
                x4 = lnp.tile([128, 4, D], BF16, tag="x4", bufs=3,
                              name=f"x4_{tname}{tg}")
                nc.sync.dma_start(
                    x4[:], xd[tname][512 * tg:512 * (tg + 1), :]
                    .rearrange("(c p) d -> p c d", p=128))
                xq4s[(tname, tg)] = x4

            xc2s = {}
            # engine for the (x - mu) subtract, per tensor
            SUB_ENG = {"q": nc.vector, "k": nc.gpsimd, "v": nc.gpsimd}

            def emit_ln(tname, t):
                tg, ti = t // 4, t % 4
                if ti == 0 and tg + 1 < 4 and (tname, tg + 1) not in xq4s:
                    emit_xload(tname, tg + 1)
                xt = xq4s[(tname, tg)][:, ti, :]
                bn6 = sml.tile([128, 6], F32, tag="bn6")
                nc.vector.bn_stats(bn6[:], xt[:])
                mv = sml.tile([128, 2], F32, tag="mv")
                nc.vector.bn_aggr(mv[:], bn6[:])
                sd = sml.tile([128, 1], F32, tag="sd")
                nc.scalar.activation(sd[:], mv[:, 1:2], AF.Sqrt, bias=epsb[:])
                nc.vector.reciprocal(rstd_all[:, TIX[tname], t:t + 1], sd[:])
                if t % 4 == 0:
                    xc2s[tname] = lnp.tile([128, 4, D], BF16, tag="z2",
                                           bufs=2, name=f"z2_{tname}{t}")
                xc2 = xc2s[tname]
                if tname == "v":
                    # -(x - mu): sign of f_v flips; y is negated at the final
                    # evacuation and b_out is pre-negated on the host.
                    nc.scalar.activation(xc2[:, t % 4, :], xt[:], AF.Identity,
                                         scale=-1.0, bias=mv[:, 0:1])
                else:
                    SUB_ENG[tname].tensor_scalar(
                        xc2[:, t % 4, :], xt[:], mv[:, 0:1], None,
                        op0=ALU.subtract)
                if t % 4 == 3:
                    nc.sync.dma_start_transpose(
                        xcTs[tname][:, :, 128 * (t // 4):128 * (t // 4 + 1)],
                        xc2[:])

            # engine for the PSUM->SBUF evacuation with rstd scale
            def emit_proj(tname, t):
                pst = ps3.tile([128, D], F32, tag="pp", name="pst")
                xcT = xcTs[tname]
                for fc in range(4):
                    nc.tensor.matmul(
                        pst[:],
                        xcT[:, 4 * (t % 4) + fc,
                            128 * (t // 4):128 * (t // 4 + 1)],
                        wp[:, fc, :],
                        start=(fc == 0), stop=(fc == 3))
                dst = fv if tname == "v" else fres[tname]
                rs = rstd_all[:, TIX[tname], t:t + 1]
                if tname == "q":
                    nc.scalar.activation(dst[:, t, :], pst[:], AF.Identity,
                                         scale=rs)
                elif tname == "k":
                    nc.scalar.activation(dst[:, t, :], pst[:], AF.Identity,
                                         scale=rs)
                else:
                    nc.vector.tensor_scalar(dst[:, t, :], pst[:], rs, None,
                                            op0=ALU.mult)

            def emit_mu(tname):
                # column-sum stats via e8b selection matmuls
                src_t = fv if tname == "v" else fres[tname]
                dst = vsum_ps if tname == "v" else mu_ps[tname]
                for t in range(NT):
                    ebb = e8b[:, 8 * (t // 2):8 * (t // 2) + 8]
                    nc.tensor.matmul(dst, ebb, src_t[:, t, :],
                                     start=(t == 0), stop=(t == NT - 1))
                if tname == "v":
                    vsb = sml.tile([8, D], F32, tag="vsb", bufs=1)
                    nc.scalar.copy(vsb[:], vsum_ps)
                    return vsb
                i = 0 if tname == "q" else 1
                mu = sml.tile([8, D], F32, tag="mu", bufs=1,
                              name=f"mu{tname}")
                nc.scalar.copy(mu[:], mu_ps[tname])
                mub = sml.tile([8, D], BF16, tag="mub", bufs=1)
                nc.scalar.copy(mub[:], mu[:])
                nc.scalar.dma_start(stb_d[2 * i], mub[:])
                mub_keep[tname] = mub
                ngb = sml.tile([8, D], BF16, tag="ngb", bufs=1)
                nc.scalar.activation(ngb[:], mu[:], AF.Identity,
                                     scale=-1.0 / N)
                nc.scalar.dma_start(stb_d[2 * i + 1], ngb[:])

            fqp4s = [None]

            def emit_cons(tname, t):
                # norms + normalized copies (pairwise over tiles)
                if t % 2 == 0:
                    return
                fb2 = fres[tname][:, t - 1:t + 1, :]
                fq2 = st3.tile([128, 2, D], BF16, tag="fsq", bufs=2,
                               name=f"fsq_{tname}{t}")
                if tname == "q":
                    nc.vector.tensor_tensor(fq2[:], fb2, fb2, op=ALU.mult)
                else:
                    nc.scalar.square(fq2[:], fb2)
                ebb = e8b[:, 8 * (t // 2):8 * (t // 2) + 8]
                for u in range(2):
                    nc.tensor.matmul(cs_ps[tname], ebb, fq2[:, u, :],
                                     start=(t == 1 and u == 0),
                                     stop=(t == NT - 1 and u == 1))
                n2 = sml.tile([128, 2, 8], F32, tag="n2")
                nc.vector.tensor_reduce(
                    n2[:], fq2[:].rearrange("p u (h e) -> p u h e", h=8),
                    axis=AX.X, op=ALU.add)
                nn = sml.tile([128, 2, 8], F32, tag="nn")
                nc.scalar.activation(nn[:].rearrange("p u h -> p (u h)"),
                                     n2[:].rearrange("p u h -> p (u h)"),
                                     AF.Sqrt,
                                     scale=(q_rs if tname == "q" else 1.0))
                niv = sml.tile([128, 2, 8], F32, tag="niv")
                nc.vector.reciprocal(
                    niv[:].rearrange("p u h -> p (u h)"),
                    nn[:].rearrange("p u h -> p (u h)"))
                if tname == "q":
                    if t % 4 == 1:
                        fqp4s[0] = st2.tile([128, 4, D], BF16, tag="fqp",
                                            bufs=2, name=f"fqp{t}")
                    dst2 = fqp4s[0][:, 2 * ((t // 2) % 2):
                                    2 * ((t // 2) % 2) + 2, :]
                else:
                    dst2 = fkn[:, t - 1:t + 1, :]
                nrm_eng = nc.gpsimd
                nrm_eng.tensor_tensor(
                    dst2.rearrange("p u (h e) -> p u h e", h=8),
                    fb2.rearrange("p u (h e) -> p u h e", h=8),
                    niv[:].rearrange("p u (h o) -> p u h o", o=1)
                          .broadcast_to([128, 2, 8, 64]),
                    op=ALU.mult)
                if tname == "q" and t % 4 == 3:
                    nc.sync.dma_start_transpose(
                        fqT[:, :, 128 * (t // 4):128 * (t // 4 + 1)],
                        fqp4s[0][:])

            def emit_strow(tname):
                i = 0 if tname == "q" else 1
                strowb = big.tile([1, 2 * Bs * D], BF16, tag="strowb",
                                  bufs=1, name=f"strowb_{tname}")
                nc.scalar.dma_start(
                    strowb[:],
                    stb_d[2 * i:2 * i + 2]
                    .rearrange("a b c -> (a b c)")[None, :])
                strow[tname] = strowb

            gram_ps = {}

            def emit_gram_mm(tname, b):
                strowb = strow[tname]

                def murow(bb):
                    return strowb[:, bb * D:(bb + 1) * D]

                def negmurow(bb):
                    return strowb[:, Bs * D + bb * D:Bs * D + (bb + 1) * D]

                fg = {0: fres[tname][:, 2 * b, :],
                      1: fres[tname][:, 2 * b + 1, :]}
                gps = ps.tile([64, D], F32, tag="gg", name="gps")
                gfirst = None
                for h in range(H):
                    sl = slice(64 * h, 64 * (h + 1))
                    for half in range(2):
                        mm = nc.tensor.matmul(gps[:, sl], fg[half][:, sl],
                                              fg[half][:, sl],
                                              start=(h == 0 and half == 0),
                                              stop=False)
                        if gfirst is None:
                            gfirst = mm
                        elif half == 0:
                            add_dep_helper(mm.ins, gfirst.ins, sync=False,
                                           reason="G window start")
                mus_mms = []
                for h in range(H):
                    sl = slice(64 * h, 64 * (h + 1))
                    mm = nc.tensor.matmul(gps[:, sl], murow(b)[:, sl],
                                          negmurow(b)[:, sl],
                                          start=False, stop=(h == H - 1))
                    add_dep_helper(mm.ins, gfirst.ins, sync=False,
                                   reason="G rank1")
                    mus_mms.append(mm)
                for mm in mus_mms[:-1]:
                    add_dep_helper(mus_mms[-1].ins, mm.ins, sync=False,
                                   reason="G window stop")
                gram_ps[(tname, b)] = gps

            def emit_gram_fin(tname, b):
                gps = gram_ps.pop((tname, b))
                g2 = st3.tile([64, D], BF16, tag="g2")
                nc.scalar.square(g2[:], gps[:])
                nc.tensor.matmul(gsq_ps[tname],
                                 e8b[0:64, 8 * b:8 * (b + 1)],
                                 g2[:], start=(b == 0), stop=(b == Bs - 1))

            def emit_gram_stats(tname):
                # stat48 pieces: V (from sigma), dq (=sum dv^2), gsq
                i = 0 if tname == "q" else 1
                mub = mub_keep[tname]
                musq = sml.tile([8, D], F32, tag="mu", bufs=1,
                                name=f"musq{tname}")
                nc.gpsimd.tensor_tensor(musq[:], mub[:], mub[:], op=ALU.mult)
                dvs = sml.tile([8, D], F32, tag="dvs", bufs=1,
                               name=f"dvs{tname}")
                nc.vector.scalar_tensor_tensor(
                    dvs[:], musq[:], -1.0 / N, cs_ps[tname],
                    op0=ALU.mult, op1=ALU.add)
                dsq = sml.tile([8, D], F32, tag="vsb", bufs=1,
                               name=f"dsq{tname}")
                nc.scalar.square(dsq[:], dvs[:])
                nc.vector.tensor_reduce(
                    stat48[:, 16 + 8 * i:24 + 8 * i],
                    dsq[:].rearrange("p (h e) -> p h e", h=8),
                    axis=AX.X, op=ALU.add)
                sg = sml.tile([8, D], F32, tag="mu", bufs=1,
                              name=f"sg{tname}")
                nc.scalar.activation(sg[:], dvs[:], AF.Sqrt,
                                     bias=epsc[:], scale=1.0 / (N - 1))
                nc.scalar.activation(sg[:], sg[:], AF.Relu,
                                     bias=one8[:], scale=-1.0)
                nc.vector.tensor_reduce(
                    stat48[:, 8 * i:8 * (i + 1)],
                    sg[:].rearrange("p (h e) -> p h e", h=8),
                    axis=AX.X, op=ALU.add)
                gsq = sml.tile([8, D], F32, tag="dvs", bufs=1,
                               name=f"gsq{tname}")
                nc.scalar.copy(gsq[:], gsq_ps[tname])
                nc.vector.tensor_reduce(
                    stat48[:, 32 + 8 * i:40 + 8 * i],
                    gsq[:].rearrange("p (h e) -> p h e", h=8),
                    axis=AX.X, op=ALU.add)

            def emit_kv_mm(b):
                kvps = ps.tile([64, D], F32, tag="gg", name="kvps")
                kvfirst, kvlast = None, []
                for h in range(H):
                    sl = slice(64 * h, 64 * (h + 1))
                    for half in range(2):
                        t = 2 * b + half
                        mm = nc.tensor.matmul(kvps[:, sl], fkn[:, t, sl],
                                              fv[:, t, sl],
                                              start=(h == 0 and half == 0),
                                              stop=(h == H - 1 and half == 1))
                        if kvfirst is None:
                            kvfirst = mm
                        else:
                            add_dep_helper(mm.ins, kvfirst.ins, sync=False,
                                           reason="KV window start")
                        if half == 1:
                            kvlast.append(mm)
                for mm in kvlast[:-1]:
                    add_dep_helper(kvlast[-1].ins, mm.ins, sync=False,
                                   reason="KV window stop")
                return kvps

            def emit_kv_fin(b, kvps):
                kv2 = st2.tile([64, D], BF16, tag="kv2")
                nc.scalar.activation(kv2[:], kvps[:], AF.Identity,
                                     scale=float(sgn))
                # block-diagonal head-pair KV: kvblk[:,b,j] = diag(KV2j, KV2j+1)
                kb = kvblk[:, b, :, :]
                nc.gpsimd.tensor_copy(
                    kb[0:64, :, 0:64],
                    kv2[0:64, :].rearrange("p (j c) -> p j c", j=4)[:, :, 0:64])
                nc.gpsimd.tensor_copy(
                    kb[64:128, :, 64:128],
                    kv2[0:64, :].rearrange("p (j o) -> p j o", j=4)[:, :, 64:128])

            outT = None

            def emit_s(b):
                fqTb = (fqT[:, :, 128 * (b // 2):128 * (b // 2 + 1)]
                        .rearrange("p (u s) c -> p s u c", s=4)
                        [:, :, 2 * (b % 2):2 * (b % 2) + 2, :])
                for jj in range(2):
                    spsT = pss.tile([128, 2, 256], F32, tag="sT", name="spsT")
                    for dj in range(2):
                        j = 2 * jj + dj
                        nc.tensor.matmul(
                            spsT[:, dj, :],
                            kvblk[:, b, j, :],
                            fqTb[:, j, :, :],
                            start=True, stop=True)
                    for hh in range(2):
                        dst = outT[:, 4 * hh + 2 * jj:4 * hh + 2 * jj + 2,
                                   128 * b:128 * (b + 1)]
                        src = spsT[:, :, 128 * hh:128 * (hh + 1)]
                        if hh == 0:
                            nc.scalar.copy(dst, src)
                        else:
                            nc.vector.tensor_copy(dst, src)

            def emit_outproj(t, yrow):
                yps = ps3.tile([128, D], F32, tag="pp", name="yps")
                for fc in range(4):
                    nc.tensor.matmul(
                        yps[:],
                        outT[:, 4 * (t % 2) + fc,
                             128 * (t // 2):128 * (t // 2 + 1)],
                        wo[:, fc, :], start=(fc == 0), stop=False)
                # rank-1: per-b additive row (stats const + b_out)
                b = t // 2
                nc.tensor.matmul(yps[:], ones1[:],
                                 yrow[:, D * b:D * (b + 1)],
                                 start=False, stop=True)
                y1 = st2.tile([128, D], BF16, tag="y4", bufs=4,
                              name=f"y1_{t}")
                if t % 2 == 0:
                    nc.scalar.activation(y1[:], yps[:], AF.Identity,
                                         scale=-1.0)
                else:
                    nc.vector.tensor_scalar_mul(y1[:], yps[:], -1.0)
                nc.sync.dma_start(y_d[128 * t:128 * (t + 1), :], y1[:])

            # ============ emission schedule ============
            # slot A: LN(q)
            emit_xload("q", 0)
            emit_xload("q", 1)
            xcTs["q"] = ztp.tile([128, 16, R // 4], BF16, tag="zT", name="zTq")
            for t in range(NT):
                emit_ln("q", t)
            emit_consts()
            nc.gpsimd.memset(kvblk[:], 0.0)
            # slot B: proj(q) | LN(k)
            emit_xload("k", 0)
            xcTs["k"] = ztp.tile([128, 16, R // 4], BF16, tag="zT", name="zTk")
            for t in range(NT):
                emit_proj("q", t)
                emit_ln("k", t)
            emit_mu("q")
            emit_strow("q")
            # slot C: proj(k) | LN(v) | cons(q) | gram(q), all interleaved
            emit_xload("v", 0)
            xcTs["v"] = ztp.tile([128, 16, R // 4], BF16, tag="zT", name="zTv")
            for p in range(Bs):
                emit_proj("k", 2 * p)
                emit_ln("v", 2 * p)
                emit_proj("k", 2 * p + 1)
                emit_ln("v", 2 * p + 1)
                emit_cons("q", 2 * p + 1)
                emit_gram_mm("q", p)
                if p > 0:
                    emit_gram_fin("q", p - 1)
            emit_gram_fin("q", Bs - 1)
            emit_gram_stats("q")
            emit_mu("k")
            emit_strow("k")
            # slot D: proj(v) | cons(k) | gram(k), interleaved; then KV
            for p in range(Bs):
                emit_proj("v", 2 * p)
                emit_cons("k", 2 * p)
                emit_proj("v", 2 * p + 1)
                emit_cons("k", 2 * p + 1)
                emit_gram_mm("k", p)
                if p > 0:
                    emit_gram_fin("k", p - 1)
            emit_gram_fin("k", Bs - 1)
            emit_gram_stats("k")
            vsb = emit_mu("v")
            kv_prev = None
            for b in range(Bs):
                kvps = emit_kv_mm(b)
                if kv_prev is not None:
                    emit_kv_fin(b - 1, kv_prev)
                kv_prev = kvps
            emit_kv_fin(Bs - 1, kv_prev)

            # chunk reduction over b (partition all-reduce on gpsimd)
            redf = sml.tile([8, 48], F32, tag="redf", bufs=1)
            nc.gpsimd.partition_all_reduce(redf[:], stat48[:], channels=8,
                                           reduce_op=bass_isa.ReduceOp.add)
            red = redf[0:1, :]
            # vc8 = var_w*Vq*Vk/(Bs*d)^2 + cov_w*Cq*Ck/(Bs*d*(N-1)^2)^2
            vv = sml.tile([1, 8], F32, tag="vv", bufs=1)
            nc.vector.scalar_tensor_tensor(
                vv[:], red[:, 0:8], float(var_w / (Bs * d) ** 2),
                red[:, 8:16], op0=ALU.mult, op1=ALU.mult)
            cdq = sml.tile([1, 8], F32, tag="cdq", bufs=1)
            nc.vector.tensor_tensor(cdq[:], red[:, 32:40], red[:, 16:24],
                                    op=ALU.subtract)
            cdk = sml.tile([1, 8], F32, tag="cdk", bufs=1)
            nc.vector.tensor_tensor(cdk[:], red[:, 40:48], red[:, 24:32],
                                    op=ALU.subtract)
            vc8 = sml.tile([1, 8], F32, tag="vc8", bufs=1)
            nc.vector.scalar_tensor_tensor(
                vc8[:], cdq[:], float(cov_w / (Bs * d * (N - 1) ** 2) ** 2),
                cdk[:], op0=ALU.mult, op1=ALU.mult)
            nc.vector.tensor_tensor(vc8[:], vv[:], vc8[:], op=ALU.add)
            # wbs[b, c] = vc8[head(c)] * vsum[b, c]
            vc8b = sml.tile([8, 8], F32, tag="vc8b", bufs=1)
            nc.gpsimd.partition_broadcast(vc8b[:], vc8[:], channels=8)
            wbs = sml.tile([8, D], BF16, tag="wbs", bufs=1)
            nc.vector.tensor_tensor(
                wbs[:].rearrange("p (h e) -> p h e", h=8),
                vsb[:].rearrange("p (h e) -> p h e", h=8),
                vc8b[:].rearrange("p (h o) -> p h o", o=1)
                       .broadcast_to([8, 8, 64]),
                op=ALU.mult)
            nc.scalar.dma_start(wb_d[:], wbs[:])
            wbT = sml.tile([128, 4, 8], BF16, tag="wbT", bufs=1)
            for j in range(4):
                nc.scalar.dma_start(
                    wbT[:, j, :],
                    wb_d[:, 128 * j:128 * (j + 1)].rearrange("b p -> p b"))
            # yadd8[b,:] = wbs[b,:] @ Wout + b_out   (the additive row per b)
            for j in range(4):
                nc.tensor.matmul(yadd_ps, wbT[:, j, :], wo[:, j, :],
                                 start=(j == 0), stop=False)
            one8r = cst.tile([1, 8], BF16, tag="one8r")
            nc.gpsimd.memset(one8r[:], 1.0)
            nc.tensor.matmul(yadd_ps, one8r[:], bob[:],
                             start=False, stop=True)
            yadd_sb = sml.tile([8, D], BF16, tag="yadd", bufs=1)
            nc.scalar.copy(yadd_sb[:], yadd_ps)
            nc.scalar.dma_start(ya_d[:], yadd_sb[:])
            yrow = big.tile([1, Bs * D], BF16, tag="yrow", bufs=1)
            nc.scalar.dma_start(
                yrow[:], ya_d[:].rearrange("b c -> (b c)")[None, :])

            # ====== Phase 3: S^T -> outT, then output projection ===========
            outT = ztp.tile([128, 8, R // 2], BF16, tag="zT", name="outT")
            emit_s(0)
            emit_s(1)
            for b in range(2, Bs):
                emit_s(b)
                emit_outproj(2 * (b - 2), yrow)
                emit_outproj(2 * (b - 2) + 1, yrow)
            for t in range(2 * (Bs - 2), NT):
                emit_outproj(t, yrow)

    nc.compile()
    return nc


def _np_fallback(q, k, v, ln_gamma, ln_beta, W_in, W_out, b_out,
                 cov_logit, var_logit):
    """use_advanced == 0 branch, plain numpy (full covariance attention)."""
    def ln(x):
        mu = x.mean(-1, keepdims=True)
        va = ((x - mu) ** 2).mean(-1, keepdims=True)
        return (x - mu) / np.sqrt(va + LN_EPS) * ln_gamma + ln_beta

    def tr(x):
        f = ln(x) @ W_in
        return f.reshape(B, N, H, d).transpose(2, 0, 1, 3)

    fq, fk, fvv = tr(q), tr(k), tr(v)
    dots = np.einsum("hbnd,hbmd->hbnm", fq, fk)
    qn = np.linalg.norm(fq, axis=-1)
    kn = np.linalg.norm(fk, axis=-1)
    cos = dots / (qn[..., :, None] * kn[..., None, :])
    qc = fq - fq.mean(-1, keepdims=True)
    kc = fk - fk.mean(-1, keepdims=True)
    cov = np.einsum("hbnd,hbmd->hbnm", qc, kc) / d
    qv = fq.var(-1, ddof=1, keepdims=True)
    kv_ = fk.var(-1, ddof=1, keepdims=True)
    var = (qv * np.swapaxes(kv_, -1, -2)) / d
    cw = 1 / (1 + np.exp(-cov_logit))
    vw = 1 / (1 + np.exp(-var_logit))
    dots = (1 - cw - vw) * cos + cw * cov + vw * var
    out = np.einsum("hbnm,hbmd->hbnd", dots, fvv)
    out = out.transpose(1, 2, 0, 3).reshape(B, N, H * d)
    return (out @ W_out + b_out).astype(np.float32)


def kernel(q, k, v, ln_gamma, ln_beta, W_in, W_out, b_out, cov_logit,
           var_logit, use_advanced):
    q = np.asarray(q, np.float32)
    k = np.asarray(k, np.float32)
    v = np.asarray(v, np.float32)
    ln_gamma = np.asarray(ln_gamma, np.float32)
    ln_beta = np.asarray(ln_beta, np.float32)
    W_in = np.asarray(W_in, np.float32)
    W_out = np.asarray(W_out, np.float32)
    b_out = np.asarray(b_out, np.float32)
    cov_f = float(np.asarray(cov_logit))
    var_f = float(np.asarray(var_logit))
    if not int(np.asarray(use_advanced)):
        return _np_fallback(q, k, v, ln_gamma, ln_beta, W_in, W_out, b_out,
                            cov_f, var_f)

    cov_w = 1.0 / (1.0 + math.exp(-cov_f))
    var_w = 1.0 / (1.0 + math.exp(-var_f))
    cos_w = 1.0 - cov_w - var_w
    Wp = (ln_gamma[:, None] * W_in + 0.0).astype(ml_dtypes.bfloat16)
    Wo = W_out.astype(ml_dtypes.bfloat16)
    if np.abs(ln_beta).max() > 0:
        # nonzero LN beta shifts f by a constant row, which changes the stats
        # nonlinearly; not modeled on-device
        return _np_advanced(q, k, v, ln_gamma, ln_beta, W_in, W_out, b_out,
                            cov_f, var_f)
    e8 = np.zeros((128, 64), np.float32)
    for b in range(8):
        e8[:, 8 * b + b] = 1.0
    key = (round(cos_w, 8), round(cov_w, 8), round(var_w, 8))
    if key not in _CACHE:
        _CACHE[key] = _build(cos_w, cov_w, var_w)
    nc = _CACHE[key]

    in_maps = []
    for c in range(NCORES):
        sl = slice(c * Bs, (c + 1) * Bs)
        in_maps.append({
            "xq": np.ascontiguousarray(
                q[sl].reshape(R, D)).astype(ml_dtypes.bfloat16),
            "xk": np.ascontiguousarray(
                k[sl].reshape(R, D)).astype(ml_dtypes.bfloat16),
            "xv": np.ascontiguousarray(
                v[sl].reshape(R, D)).astype(ml_dtypes.bfloat16),
            "wp": Wp, "wo": Wo,
            "bob": (-b_out).reshape(1, D).astype(ml_dtypes.bfloat16),
            "e8b": e8.astype(ml_dtypes.bfloat16),
        })
    res = run_bass_kernel_spmd(nc, in_maps, list(range(NCORES)))
    y = np.stack([np.asarray(res.results[c]["y"], np.float32).reshape(Bs, N, D)
                  for c in range(NCORES)], 0).reshape(B, N, D)
    return y


def _np_advanced(q, k, v, ln_gamma, ln_beta, W_in, W_out, b_out,
                 cov_logit, var_logit):
    """numpy advanced branch (general beta), mirrors reference exactly."""
    def ln(x):
        mu = x.mean(-1, keepdims=True)
        va = ((x - mu) ** 2).mean(-1, keepdims=True)
        return (x - mu) / np.sqrt(va + LN_EPS) * ln_gamma + ln_beta

    def tr(x):
        f = ln(x) @ W_in
        return f.reshape(B, N, H, d).transpose(2, 0, 1, 3)

    CH = 8
    fq, fk, fvv = tr(q), tr(k), tr(v)
    dots = np.einsum("hbnd,hbmd->hbnm", fq, fk)
    qn = np.linalg.norm(fq, axis=-1)
    kn = np.linalg.norm(fk, axis=-1)
    cos = dots / (qn[..., :, None] * kn[..., None, :])

    def chunk_stats(E):
        M, n, dd = E.shape
        nch = M // CH
        Ec = E.reshape(nch, CH, n, dd)
        sig = np.sqrt(Ec.var(2, ddof=1) + EPS)
        V = np.maximum(1.0 - sig, 0).mean((1, 2))
        cen = Ec - Ec.mean(2, keepdims=True)
        cov = np.einsum("csni,csnj->csij", cen, cen) / (n - 1)
        tot = (cov ** 2).sum((2, 3))
        dia = (np.einsum("csii->csi", cov) ** 2).sum(2)
        return V, ((tot - dia) / dd).mean(1)

    Vq, Cq = chunk_stats(fq.reshape(H * B, N, d))
    Vk, Ck = chunk_stats(fk.reshape(H * B, N, d))
    var_c = np.repeat(Vq * Vk, CH).reshape(H, B, 1, 1)
    cov_c = np.repeat(Cq * Ck, CH).reshape(H, B, 1, 1)
    cw = 1 / (1 + np.exp(-cov_logit))
    vw = 1 / (1 + np.exp(-var_logit))
    dots = (1 - cw - vw) * cos + cw * cov_c + vw * var_c
    out = np.einsum("hbnm,hbmd->hbnd", dots, fvv)
    out = out.transpose(1, 2, 0, 3).reshape(B, N, H * d)
    return (out @ W_out + b_out).astype(np.float32)
